# revision 64
# baseline (speedup 1.0000x reference)
"""MoE (top-2 of 32 experts, 512->512) on 8 NeuronCores, expert-parallel.

Strategy (full-I/O contract: kernel() receives full inputs, returns full output):
  - Host computes the small selector (softmax gate + top-k) in fp32 numpy,
    mirroring the reference op-for-op, and performs the "all-to-all dispatch":
    tokens are grouped by expert into capacity-padded batches (the host-side
    sharding step of the expert-parallel layout).
  - Experts are sharded 4-per-core across the 8 cores.  Each core runs a
    Bass kernel computing, per expert, relu((w*x) @ We + w*be) with weights
    STATIONARY and tokens as the moving operand, so slot capacities are
    token-granular (16) instead of 128-padded.  The gate weight w >= 0 is
    pre-multiplied into x on the host (w*relu(z) == relu(w*z)); a nonzero
    bias is added pre-relu as the rank-1 matmul be (x) w.
  - Host combines: out[token] = sum of its k (device-weighted) slot columns.

Precision modes for the expert GEMMs (MODE):
  - "f32"  : exact fp32 matmuls (PE runs them as 2 half-rate passes).
  - "f16x3": x and W split into fp16 hi+lo; y = xh@wh + xh@wl + xl@wh.
             fp16 products are exact in fp32 (11+11 mantissa bits < 24), so
             the only extra error is the dropped xl@wl term (~2^-22 relative)
             - fp32-equivalent accuracy at 3/4 the PE cost of "f32".
  - "f32r" : single-pass relaxed-precision fp32 (tf32-like, ~1e-4 rel err).
  - "bf16" : single-pass bf16 (~1e-3 rel err).

All tensors are pre-swizzled on the host into the exact SBUF layout so every
device DMA is a contiguous copy.  The device kernel is compiled once per
(capacity, mode, has_bias) and cached.
"""

import numpy as np
from contextlib import ExitStack

B, NIN, NOUT, E, NCORES = 8192, 512, 512, 32, 8
EPC = E // NCORES  # experts per core
KCH = NIN // 128   # contraction chunks of 128

MODE = "f16x3"

_CACHE = {}


def _mode_spec(mode):
    import concourse.mybir as mybir
    import ml_dtypes
    if mode == "f32":
        return dict(dt=mybir.dt.float32, npdt=np.float32, nsplit=1,
                    passes=[(0, 0)])
    if mode == "f32r":
        return dict(dt=mybir.dt.float32r, npdt=np.float32, nsplit=1,
                    passes=[(0, 0)])
    if mode == "bf16":
        return dict(dt=mybir.dt.bfloat16, npdt=ml_dtypes.bfloat16, nsplit=1,
                    passes=[(0, 0)])
    if mode == "f16x3":
        return dict(dt=mybir.dt.float16, npdt=np.float16, nsplit=2,
                    passes=[(0, 0), (0, 1), (1, 0)])
    raise ValueError(mode)


def _build(caps, mode, has_bias):
    """Build + compile the per-core Bass program.  caps[j] = token capacity
    (multiple of 16) of expert slot j - same structure on every core; the
    host assigns its busiest expert to slot 0 etc. so capacity is not
    wasted padding every expert to the global max."""
    import concourse.mybir as mybir
    import concourse.tile as tile
    from concourse import bacc

    spec = _mode_spec(mode)
    mmdt = spec["dt"]
    nsplit = spec["nsplit"]
    passes = spec["passes"]
    Cj = list(caps)                        # tokens per slot (multiple of 16)
    CT = sum(Cj)                           # tokens per core
    ROFF = np.concatenate([[0], np.cumsum(Cj)]).astype(int)   # token offsets
    XOFF = [KCH * r for r in ROFF]         # x free-elem offsets per slot
    XF = KCH * CT           # x free elems per split: [p][slot][kc][c]
    WF = EPC * KCH * NOUT   # w free elems per split: [p][slot][kc][n]

    nc = bacc.Bacc("TRN2", target_bir_lowering=False, debug=False,
                   num_devices=NCORES)
    # hi/lo splits are stacked along the free axis of ONE tensor so a single
    # DMA moves both (fewer DMA triggers - they serialize on the HWDGE ring)
    xd = nc.dram_tensor("xd", [128, nsplit * XF], mmdt,
                        kind="ExternalInput").ap()
    wd = nc.dram_tensor("wd", [128, nsplit * WF], mmdt,
                        kind="ExternalInput").ap()
    if has_bias:
        cw = nc.dram_tensor("cw", [1, CT], mybir.dt.float32,
                            kind="ExternalInput").ap()
        be = nc.dram_tensor("be", [1, EPC * NOUT], mybir.dt.float32,
                            kind="ExternalInput").ap()
    # out is transposed: [nout, tokens] (weights are the stationary operand)
    out = nc.dram_tensor("out", [NOUT, CT], mybir.dt.float32,
                         kind="ExternalOutput").ap()

    NS = NOUT // 128                       # nout slices
    # token chunks per slot (moving-N <= 512 per matmul / one PSUM bank);
    # taper carves a 64-token final chunk so the last epilogue is tiny and
    # the big chunk's ACT+DMA chain overlaps the taper's matmuls
    def chunks_of(c, taper=False):
        res, o = [], 0
        while o < c:
            n = min(512, c - o)
            res.append((o, n))
            o += n
        return res

    with tile.TileContext(nc) as tc, ExitStack() as ctx:
        xpool = ctx.enter_context(tc.tile_pool(name="x", bufs=1))
        wpool = ctx.enter_context(tc.tile_pool(name="w", bufs=1))
        spool = ctx.enter_context(tc.tile_pool(name="s", bufs=1))
        opool = ctx.enter_context(tc.tile_pool(name="o", bufs=1))
        pspool = ctx.enter_context(tc.tile_pool(name="ps", bufs=2,
                                                space="PSUM"))

        x_all = xpool.tile([128, nsplit * XF], mmdt, name="x_all")
        w_all = wpool.tile([128, nsplit * WF], mmdt, name="w_all")
        x_sb = [x_all[:, s * XF:(s + 1) * XF] for s in range(nsplit)]
        w_sb = [w_all[:, s * WF:(s + 1) * WF] for s in range(nsplit)]

        if has_bias:
            # tiny and needed by every epilogue: first on the FIFO ring
            cw_sb = spool.tile([1, CT], mybir.dt.float32)
            nc.sync.dma_start(cw_sb[:], cw)
            bias_sb = spool.tile([1, EPC * NOUT], mybir.dt.float32)
            nc.sync.dma_start(bias_sb[:], be)

        xv = xd.rearrange("p (s f) -> p s f", s=nsplit)
        wv = wd.rearrange("p (s f) -> p s f", s=nsplit)
        xav = x_all[:].rearrange("p (s f) -> p s f", s=nsplit)
        wav = w_all[:].rearrange("p (s f) -> p s f", s=nsplit)
        for j in range(EPC):
            if caps[j] == 0:
                continue
            # one DMA covers hi+lo; finer chunks for slot 0 so the first
            # matmul starts sooner.  Slot 0's first k-chunk additionally
            # ships hi before lo, in the order the (kc, pass) matmul stream
            # consumes it: wh, xh (pass 1), wl (pass 2), xl (pass 3).
            if j == 0 and nsplit == 2:
                # k-chunk 0 ships hi before lo in the order the (kc, pass,
                # ns) matmul stream consumes it: wh, xh, wl, xl; k-chunks
                # 1-3 follow via the generic loop below
                for (s, a_, v_, l_, r_) in (
                        (0, wav, wv, 0, NOUT),
                        (0, xav, xv, XOFF[0], XOFF[0] + Cj[0]),
                        (1, wav, wv, 0, NOUT),
                        (1, xav, xv, XOFF[0], XOFF[0] + Cj[0])):
                    nc.sync.dma_start(a_[:, s:s + 1, l_:r_],
                                      v_[:, s:s + 1, l_:r_])
            KH = 1 if j == 0 else 2
            for kh in range(KH if j == 0 else 0, KCH, KH):
                xl_ = XOFF[j] + kh * Cj[j]
                xr_ = XOFF[j] + (kh + KH) * Cj[j]
                wl_ = (j * KCH + kh) * NOUT
                wr_ = (j * KCH + kh + KH) * NOUT
                nc.sync.dma_start(xav[:, :, xl_:xr_], xv[:, :, xl_:xr_])
                nc.sync.dma_start(wav[:, :, wl_:wr_], wv[:, :, wl_:wr_])

        # out staging: [p (nout within slice)][ns][token]
        out_sb = opool.tile([128, NS * CT], mybir.dt.float32)

        # PE warmup: dependency-free dummy matmuls ramp the PE clock to full
        # speed while the first input DMAs are still in flight.
        wu_sb = spool.tile([128, 512], mmdt)
        nc.vector.memset(wu_sb[:], 0.0)
        wu_ps = pspool.tile([128, 512], mybir.dt.float32, tag="ps3")
        for _ in range(4):
            nc.tensor.matmul(wu_ps[:], lhsT=wu_sb[:, :128], rhs=wu_sb[:],
                             start=True, stop=True)

        npass = len(passes)
        nmm = KCH * npass + (1 if has_bias else 0)
        for j in range(EPC):
            if caps[j] == 0:
                continue
            for ci, (tok0, tokn) in enumerate(
                    chunks_of(Cj[j], taper=(j == EPC - 1))):
                pss = [pspool.tile([128, tokn], mybir.dt.float32,
                                   name=f"ps_{j}_{ci}_{ns}", tag=f"ps{ns}")
                       for ns in range(NS)]
                # k-chunk outer so the first matmuls need only k-chunk 0;
                # slot 0 goes pass-mid to match its hi-first DMA order
                if j == 0:
                    order = [(kc, p, ns) for kc in range(KCH)
                             for p in range(npass) for ns in range(NS)]
                else:
                    order = [(kc, p, ns) for kc in range(KCH)
                             for ns in range(NS) for p in range(npass)]
                for (kc, p, ns) in order:
                    sx, sw = passes[p]
                    m = kc * npass + p + 1
                    nc.tensor.matmul(
                        pss[ns][:],
                        lhsT=w_sb[sw][:, (j * KCH + kc) * NOUT + ns * 128:
                                      (j * KCH + kc) * NOUT + (ns + 1) * 128],
                        rhs=x_sb[sx][:, XOFF[j] + kc * Cj[j] + tok0:
                                     XOFF[j] + kc * Cj[j] + tok0 + tokn],
                        start=(m == 1), stop=(m == nmm))
                last = j == EPC - 1 and tok0 + tokn == Cj[j]
                out_eng = nc.scalar if j < EPC // 2 else nc.sync
                for ns in range(NS):
                    if has_bias:
                        # psum holds w*(x@W); add w*be via rank-1 matmul
                        nc.tensor.matmul(
                            pss[ns][:],
                            lhsT=bias_sb[:, j * NOUT + ns * 128:
                                         j * NOUT + (ns + 1) * 128],
                            rhs=cw_sb[:, ROFF[j] + tok0:
                                      ROFF[j] + tok0 + tokn],
                            start=False, stop=True)
                    off = ns * CT + ROFF[j] + tok0
                    nc.scalar.activation(
                        out_sb[:, off:off + tokn], pss[ns][:],
                        mybir.ActivationFunctionType.Relu)
                    (nc.sync if last else out_eng).dma_start(
                        out[ns * 128:(ns + 1) * 128,
                            ROFF[j] + tok0:ROFF[j] + tok0 + tokn],
                        out_sb[:, off:off + tokn])

    nc.compile()
    return nc


def _make_runner(nc):
    """One-time jit of the 8-core SPMD executable (mirrors
    bass2jax.run_bass_via_pjrt, cached so repeat calls skip retracing)."""
    import jax
    import jax.core
    import numpy as _np
    from jax.sharding import Mesh, PartitionSpec
    from jax.experimental.shard_map import shard_map
    from concourse import bass2jax, mybir

    bass2jax.install_neuronx_cc_hook()

    partition_name = (nc.partition_id_tensor.name
                      if nc.partition_id_tensor else None)
    in_names, out_names, out_avals, zero_shapes = [], [], [], []
    for alloc in nc.m.functions[0].allocations:
        if not isinstance(alloc, mybir.MemoryLocationSet):
            continue
        name = alloc.memorylocations[0].name
        if alloc.kind == "ExternalInput":
            if name != partition_name:
                in_names.append(name)
        elif alloc.kind == "ExternalOutput":
            out_names.append(name)
            shape = tuple(alloc.tensor_shape)
            dt = mybir.dt.np(alloc.dtype)
            out_avals.append(jax.core.ShapedArray(shape, dt))
            zero_shapes.append((shape, dt))
    n_params = len(in_names)
    all_names = in_names + out_names
    if partition_name is not None:
        all_names = all_names + [partition_name]

    def _body(*args):
        operands = list(args)
        if partition_name is not None:
            operands.append(bass2jax.partition_id_tensor())
        outs = bass2jax._bass_exec_p.bind(
            *operands,
            out_avals=tuple(out_avals),
            in_names=tuple(all_names),
            out_names=tuple(out_names),
            lowering_input_output_aliases=(),
            sim_require_finite=True,
            sim_require_nnan=True,
            nc=nc,
        )
        return tuple(outs)

    devices = jax.devices()[:NCORES]
    mesh = Mesh(_np.asarray(devices), ("core",))
    n_outs = len(out_names)
    specs = (PartitionSpec("core"),) * (n_params + n_outs)
    donate = tuple(range(n_params, n_params + n_outs))
    sharded = jax.jit(
        shard_map(_body, mesh=mesh, in_specs=specs,
                  out_specs=(PartitionSpec("core"),) * n_outs,
                  check_rep=False),
        donate_argnums=donate, keep_unused=True)

    def run(feeds):
        """feeds: dict name -> full concatenated array [NCORES*dim0, ...].
        Returns dict name -> full concatenated output array."""
        concat_in = [feeds[name] for name in in_names]
        concat_zeros = [
            _np.zeros((NCORES * s[0],) + tuple(s[1:]), dt)
            for (s, dt) in zero_shapes
        ]
        out_arrs = sharded(*concat_in, *concat_zeros)
        return {name: _np.asarray(out_arrs[i])
                for i, name in enumerate(out_names)}

    # exposed for benchmarking (test.py)
    run._sharded = sharded
    run._in_names = in_names
    run._zero_shapes = zero_shapes
    return run


def _get_runner(caps, mode, has_bias):
    key = (caps, mode, has_bias)
    if key not in _CACHE:
        nc = _build(caps, mode, has_bias)
        _CACHE[key] = (nc, _make_runner(nc))
    return _CACHE[key]


def _route(x, Wg, bg, k):
    """Replicates the reference selector in fp32: softmax gate, top-k
    (stable, ties to lower index like jax.lax.top_k), aux loss."""
    logits = x @ Wg + bg
    m = logits.max(-1, keepdims=True)
    p = np.exp(logits - m)
    gate = p / p.sum(-1, keepdims=True)
    idx = np.argsort(-gate, axis=-1, kind="stable")[:, :k]      # [B, k]
    vals = np.take_along_axis(gate, idx, axis=-1)               # [B, k]
    row_sum = gate.sum(-1)
    aux = (np.var(row_sum) / (np.mean(row_sum) ** 2 + np.float32(1e-10)))
    return idx, vals, np.float32(aux)


def _split_into(dst, view, spec, F):
    """Write `view` [NCORES, 128, F] (fp32, any strides) into dst
    [NCORES*128, nsplit*F] as hi (and lo residual for split modes)."""
    d = dst.reshape(NCORES, 128, -1)
    d[:, :, :F] = view                      # cast fp32 -> device dtype
    if spec["nsplit"] == 2:
        d[:, :, F:] = view - d[:, :, :F].astype(np.float32)


def _prepare(x, Wg, bg, We, be, k, mode):
    """Route + dispatch: returns (caps, has_bias, feeds, dest, inv, aux)."""
    spec = _mode_spec(mode)
    idx, vals, aux = _route(x, Wg, bg, k)

    ef = idx.ravel()
    wf = vals.ravel()
    tf = np.repeat(np.arange(B), k)
    order = np.argsort(ef, kind="stable")
    counts = np.bincount(ef, minlength=E)

    # Load-sorted slot assignment: expert with load-rank r goes to core r%8,
    # slot r//8; slot j's capacity is the max token count in rank octile j
    # (rounded to 16), so every core compiles to the same block structure.
    rank_order = np.argsort(-counts, kind="stable")      # expert ids by load
    caps = tuple(int(max(16, -(-counts[rank_order[NCORES * j]] // 16) * 16))
                 for j in range(EPC))
    Cj = np.array(caps)
    CT = int(Cj.sum())
    ROFF = np.concatenate([[0], np.cumsum(Cj)]).astype(np.int64)

    core_of = np.empty(E, dtype=np.int64)
    slot_of = np.empty(E, dtype=np.int64)
    core_of[rank_order] = np.arange(E) % NCORES
    slot_of[rank_order] = np.arange(E) // NCORES

    starts = np.zeros(E, dtype=np.int64)
    starts[1:] = np.cumsum(counts)[:-1]
    base = core_of * CT + ROFF[slot_of]                  # per-expert row base
    es = ef[order]
    dest = base[es] + (np.arange(B * k) - starts[es])    # unique global rows
    inv = np.argsort(order, kind="stable")

    # Tokens are PRE-SCALED by their gate weight (w >= 0 so
    # relu((w*x)@W + w*be) == w*relu(x@W + be)); padding rows stay zero.
    Xg = np.zeros((NCORES * CT, NIN), dtype=np.float32)
    Xg[dest] = x[tf[order]] * wf[order][:, None]
    cwg = np.zeros(NCORES * CT, dtype=np.float32)
    cwg[dest] = wf[order]

    npdt = spec["npdt"]
    ns = spec["nsplit"]
    XF = KCH * CT
    WF = EPC * KCH * NOUT

    # swizzle straight into the concat-ready device feeds
    # x: [core][p (nin within chunk)][slot][kc][token], token-granular slots
    xsw = np.empty((NCORES, 128, XF), np.float32)
    Xg3 = Xg.reshape(NCORES, CT, NIN)
    for j in range(EPC):
        xsw[:, :, KCH * ROFF[j]:KCH * ROFF[j + 1]] = (
            Xg3[:, ROFF[j]:ROFF[j + 1], :]
            .reshape(NCORES, Cj[j], KCH, 128).transpose(0, 3, 2, 1)
            .reshape(NCORES, 128, KCH * Cj[j]))
    xd = np.empty((NCORES * 128, ns * XF), npdt)
    _split_into(xd, xsw, spec, XF)

    # w: [core][p][slot][kc][n]; core c's slot j holds expert rank_order[8j+c]
    eid = rank_order.reshape(EPC, NCORES).T              # [core, slot]
    wd = np.empty((NCORES * 128, ns * WF), npdt)
    _split_into(wd, We[eid].reshape(NCORES, EPC, KCH, 128, NOUT)
                .transpose(0, 3, 1, 2, 4).reshape(NCORES, 128, WF), spec, WF)

    has_bias = bool(np.any(be))
    feeds = {"xd": xd, "wd": wd}
    if has_bias:
        feeds["cw"] = np.ascontiguousarray(cwg.reshape(NCORES, CT))
        feeds["be"] = np.ascontiguousarray(
            be[eid].reshape(NCORES, EPC * NOUT))   # [core][1, e*n] rows
    return caps, has_bias, feeds, dest, inv, aux, CT


def kernel(x, Wg, bg, We, be, k):
    x = np.ascontiguousarray(np.asarray(x, dtype=np.float32))
    Wg = np.asarray(Wg, dtype=np.float32)
    bg = np.asarray(bg, dtype=np.float32)
    We = np.ascontiguousarray(np.asarray(We, dtype=np.float32))
    be = np.ascontiguousarray(np.asarray(be, dtype=np.float32))
    k = int(k)

    caps, has_bias, feeds, dest, inv, aux, CT = _prepare(x, Wg, bg, We, be,
                                                        k, MODE)
    _, run = _get_runner(caps, MODE, has_bias)
    results = run(feeds)

    # out is [NCORES*NOUT, CT] with nout on rows (weights were stationary)
    Y3 = results["out"].reshape(NCORES, NOUT, CT)
    d = dest[inv]                                       # token-major slots
    slot_rows = Y3[d // CT, :, d % CT]                  # [B*k, NOUT]
    out = slot_rows.reshape(B, k, NOUT).sum(axis=1, dtype=np.float32)
    return out, aux


# revision 65
# speedup vs baseline: 1.0005x; 1.0005x over previous
"""MoE (top-2 of 32 experts, 512->512) on 8 NeuronCores, expert-parallel.

Strategy (full-I/O contract: kernel() receives full inputs, returns full output):
  - Host computes the small selector (softmax gate + top-k) in fp32 numpy,
    mirroring the reference op-for-op, and performs the "all-to-all dispatch":
    tokens are grouped by expert into capacity-padded batches (the host-side
    sharding step of the expert-parallel layout).
  - Experts are sharded 4-per-core across the 8 cores.  Each core runs a
    Bass kernel computing, per expert, relu((w*x) @ We + w*be) with weights
    STATIONARY and tokens as the moving operand, so slot capacities are
    token-granular (16) instead of 128-padded.  The gate weight w >= 0 is
    pre-multiplied into x on the host (w*relu(z) == relu(w*z)); a nonzero
    bias is added pre-relu as the rank-1 matmul be (x) w.
  - Host combines: out[token] = sum of its k (device-weighted) slot columns.

Precision modes for the expert GEMMs (MODE):
  - "f32"  : exact fp32 matmuls (PE runs them as 2 half-rate passes).
  - "f16x3": x and W split into fp16 hi+lo; y = xh@wh + xh@wl + xl@wh.
             fp16 products are exact in fp32 (11+11 mantissa bits < 24), so
             the only extra error is the dropped xl@wl term (~2^-22 relative)
             - fp32-equivalent accuracy at 3/4 the PE cost of "f32".
  - "f32r" : single-pass relaxed-precision fp32 (tf32-like, ~1e-4 rel err).
  - "bf16" : single-pass bf16 (~1e-3 rel err).

All tensors are pre-swizzled on the host into the exact SBUF layout so every
device DMA is a contiguous copy.  The device kernel is compiled once per
(capacity, mode, has_bias) and cached.
"""

import numpy as np
from contextlib import ExitStack

B, NIN, NOUT, E, NCORES = 8192, 512, 512, 32, 8
EPC = E // NCORES  # experts per core
KCH = NIN // 128   # contraction chunks of 128

MODE = "f16x3"

_CACHE = {}


def _mode_spec(mode):
    import concourse.mybir as mybir
    import ml_dtypes
    if mode == "f32":
        return dict(dt=mybir.dt.float32, npdt=np.float32, nsplit=1,
                    passes=[(0, 0)])
    if mode == "f32r":
        return dict(dt=mybir.dt.float32r, npdt=np.float32, nsplit=1,
                    passes=[(0, 0)])
    if mode == "bf16":
        return dict(dt=mybir.dt.bfloat16, npdt=ml_dtypes.bfloat16, nsplit=1,
                    passes=[(0, 0)])
    if mode == "f16x3":
        return dict(dt=mybir.dt.float16, npdt=np.float16, nsplit=2,
                    passes=[(0, 0), (0, 1), (1, 0)])
    raise ValueError(mode)


def _build(caps, mode, has_bias):
    """Build + compile the per-core Bass program.  caps[j] = token capacity
    (multiple of 16) of expert slot j - same structure on every core; the
    host assigns its busiest expert to slot 0 etc. so capacity is not
    wasted padding every expert to the global max."""
    import concourse.mybir as mybir
    import concourse.tile as tile
    from concourse import bacc

    spec = _mode_spec(mode)
    mmdt = spec["dt"]
    nsplit = spec["nsplit"]
    passes = spec["passes"]
    Cj = list(caps)                        # tokens per slot (multiple of 16)
    CT = sum(Cj)                           # tokens per core
    ROFF = np.concatenate([[0], np.cumsum(Cj)]).astype(int)   # token offsets
    XOFF = [KCH * r for r in ROFF]         # x free-elem offsets per slot
    XF = KCH * CT           # x free elems per split: [p][slot][kc][c]
    WF = EPC * KCH * NOUT   # w free elems per split: [p][slot][kc][n]

    nc = bacc.Bacc("TRN2", target_bir_lowering=False, debug=False,
                   num_devices=NCORES)
    # hi/lo splits are stacked along the free axis of ONE tensor so a single
    # DMA moves both (fewer DMA triggers - they serialize on the HWDGE ring)
    xd = nc.dram_tensor("xd", [128, nsplit * XF], mmdt,
                        kind="ExternalInput").ap()
    wd = nc.dram_tensor("wd", [128, nsplit * WF], mmdt,
                        kind="ExternalInput").ap()
    if has_bias:
        cw = nc.dram_tensor("cw", [1, CT], mybir.dt.float32,
                            kind="ExternalInput").ap()
        be = nc.dram_tensor("be", [1, EPC * NOUT], mybir.dt.float32,
                            kind="ExternalInput").ap()
    # out is transposed: [nout, tokens] (weights are the stationary operand)
    out = nc.dram_tensor("out", [NOUT, CT], mybir.dt.float32,
                         kind="ExternalOutput").ap()

    NS = NOUT // 128                       # nout slices
    # token chunks per slot (moving-N <= 512 per matmul / one PSUM bank);
    # taper carves a 64-token final chunk so the last epilogue is tiny and
    # the big chunk's ACT+DMA chain overlaps the taper's matmuls
    def chunks_of(c, taper=False):
        res, o = [], 0
        while o < c:
            n = min(512, c - o)
            res.append((o, n))
            o += n
        return res

    with tile.TileContext(nc) as tc, ExitStack() as ctx:
        xpool = ctx.enter_context(tc.tile_pool(name="x", bufs=1))
        wpool = ctx.enter_context(tc.tile_pool(name="w", bufs=1))
        spool = ctx.enter_context(tc.tile_pool(name="s", bufs=1))
        opool = ctx.enter_context(tc.tile_pool(name="o", bufs=1))
        pspool = ctx.enter_context(tc.tile_pool(name="ps", bufs=2,
                                                space="PSUM"))

        x_all = xpool.tile([128, nsplit * XF], mmdt, name="x_all")
        w_all = wpool.tile([128, nsplit * WF], mmdt, name="w_all")
        x_sb = [x_all[:, s * XF:(s + 1) * XF] for s in range(nsplit)]
        w_sb = [w_all[:, s * WF:(s + 1) * WF] for s in range(nsplit)]

        if has_bias:
            # tiny and needed by every epilogue: first on the FIFO ring
            cw_sb = spool.tile([1, CT], mybir.dt.float32)
            nc.sync.dma_start(cw_sb[:], cw)
            bias_sb = spool.tile([1, EPC * NOUT], mybir.dt.float32)
            nc.sync.dma_start(bias_sb[:], be)

        xv = xd.rearrange("p (s f) -> p s f", s=nsplit)
        wv = wd.rearrange("p (s f) -> p s f", s=nsplit)
        xav = x_all[:].rearrange("p (s f) -> p s f", s=nsplit)
        wav = w_all[:].rearrange("p (s f) -> p s f", s=nsplit)
        for j in range(EPC):
            if caps[j] == 0:
                continue
            # one DMA covers hi+lo; finer chunks for slot 0 so the first
            # matmul starts sooner.  Slot 0's first k-chunk additionally
            # ships hi before lo, in the order the (kc, pass) matmul stream
            # consumes it: wh, xh (pass 1), wl (pass 2), xl (pass 3).
            if j == 0 and nsplit == 2:
                # k-chunk 0 ships hi before lo in the order the (kc, pass,
                # ns) matmul stream consumes it: wh, xh, wl, xl; k-chunks
                # 1-3 follow via the generic loop below
                for (s, a_, v_, l_, r_) in (
                        (0, wav, wv, 0, NOUT),
                        (0, xav, xv, XOFF[0], XOFF[0] + Cj[0]),
                        (1, wav, wv, 0, NOUT),
                        (1, xav, xv, XOFF[0], XOFF[0] + Cj[0])):
                    nc.sync.dma_start(a_[:, s:s + 1, l_:r_],
                                      v_[:, s:s + 1, l_:r_])
            KH = 1 if j == 0 else 4
            for kh in range(KH if j == 0 else 0, KCH, KH):
                xl_ = XOFF[j] + kh * Cj[j]
                xr_ = XOFF[j] + (kh + KH) * Cj[j]
                wl_ = (j * KCH + kh) * NOUT
                wr_ = (j * KCH + kh + KH) * NOUT
                nc.sync.dma_start(xav[:, :, xl_:xr_], xv[:, :, xl_:xr_])
                nc.sync.dma_start(wav[:, :, wl_:wr_], wv[:, :, wl_:wr_])

        # out staging: [p (nout within slice)][ns][token]
        out_sb = opool.tile([128, NS * CT], mybir.dt.float32)

        # PE warmup: dependency-free dummy matmuls ramp the PE clock to full
        # speed while the first input DMAs are still in flight.
        wu_sb = spool.tile([128, 512], mmdt)
        nc.vector.memset(wu_sb[:], 0.0)
        wu_ps = pspool.tile([128, 512], mybir.dt.float32, tag="ps3")
        for _ in range(4):
            nc.tensor.matmul(wu_ps[:], lhsT=wu_sb[:, :128], rhs=wu_sb[:],
                             start=True, stop=True)

        npass = len(passes)
        nmm = KCH * npass + (1 if has_bias else 0)
        for j in range(EPC):
            if caps[j] == 0:
                continue
            for ci, (tok0, tokn) in enumerate(
                    chunks_of(Cj[j], taper=(j == EPC - 1))):
                pss = [pspool.tile([128, tokn], mybir.dt.float32,
                                   name=f"ps_{j}_{ci}_{ns}", tag=f"ps{ns}")
                       for ns in range(NS)]
                # k-chunk outer so the first matmuls need only k-chunk 0;
                # slot 0 goes pass-mid to match its hi-first DMA order
                if j == 0:
                    order = [(kc, p, ns) for kc in range(KCH)
                             for p in range(npass) for ns in range(NS)]
                else:
                    order = [(kc, p, ns) for kc in range(KCH)
                             for ns in range(NS) for p in range(npass)]
                for (kc, p, ns) in order:
                    sx, sw = passes[p]
                    m = kc * npass + p + 1
                    nc.tensor.matmul(
                        pss[ns][:],
                        lhsT=w_sb[sw][:, (j * KCH + kc) * NOUT + ns * 128:
                                      (j * KCH + kc) * NOUT + (ns + 1) * 128],
                        rhs=x_sb[sx][:, XOFF[j] + kc * Cj[j] + tok0:
                                     XOFF[j] + kc * Cj[j] + tok0 + tokn],
                        start=(m == 1), stop=(m == nmm))
                last = j == EPC - 1 and tok0 + tokn == Cj[j]
                out_eng = nc.scalar if j < EPC // 2 else nc.sync
                for ns in range(NS):
                    if has_bias:
                        # psum holds w*(x@W); add w*be via rank-1 matmul
                        nc.tensor.matmul(
                            pss[ns][:],
                            lhsT=bias_sb[:, j * NOUT + ns * 128:
                                         j * NOUT + (ns + 1) * 128],
                            rhs=cw_sb[:, ROFF[j] + tok0:
                                      ROFF[j] + tok0 + tokn],
                            start=False, stop=True)
                    off = ns * CT + ROFF[j] + tok0
                    nc.scalar.activation(
                        out_sb[:, off:off + tokn], pss[ns][:],
                        mybir.ActivationFunctionType.Relu)
                    (nc.sync if last else out_eng).dma_start(
                        out[ns * 128:(ns + 1) * 128,
                            ROFF[j] + tok0:ROFF[j] + tok0 + tokn],
                        out_sb[:, off:off + tokn])

    nc.compile()
    return nc


def _make_runner(nc):
    """One-time jit of the 8-core SPMD executable (mirrors
    bass2jax.run_bass_via_pjrt, cached so repeat calls skip retracing)."""
    import jax
    import jax.core
    import numpy as _np
    from jax.sharding import Mesh, PartitionSpec
    from jax.experimental.shard_map import shard_map
    from concourse import bass2jax, mybir

    bass2jax.install_neuronx_cc_hook()

    partition_name = (nc.partition_id_tensor.name
                      if nc.partition_id_tensor else None)
    in_names, out_names, out_avals, zero_shapes = [], [], [], []
    for alloc in nc.m.functions[0].allocations:
        if not isinstance(alloc, mybir.MemoryLocationSet):
            continue
        name = alloc.memorylocations[0].name
        if alloc.kind == "ExternalInput":
            if name != partition_name:
                in_names.append(name)
        elif alloc.kind == "ExternalOutput":
            out_names.append(name)
            shape = tuple(alloc.tensor_shape)
            dt = mybir.dt.np(alloc.dtype)
            out_avals.append(jax.core.ShapedArray(shape, dt))
            zero_shapes.append((shape, dt))
    n_params = len(in_names)
    all_names = in_names + out_names
    if partition_name is not None:
        all_names = all_names + [partition_name]

    def _body(*args):
        operands = list(args)
        if partition_name is not None:
            operands.append(bass2jax.partition_id_tensor())
        outs = bass2jax._bass_exec_p.bind(
            *operands,
            out_avals=tuple(out_avals),
            in_names=tuple(all_names),
            out_names=tuple(out_names),
            lowering_input_output_aliases=(),
            sim_require_finite=True,
            sim_require_nnan=True,
            nc=nc,
        )
        return tuple(outs)

    devices = jax.devices()[:NCORES]
    mesh = Mesh(_np.asarray(devices), ("core",))
    n_outs = len(out_names)
    specs = (PartitionSpec("core"),) * (n_params + n_outs)
    donate = tuple(range(n_params, n_params + n_outs))
    sharded = jax.jit(
        shard_map(_body, mesh=mesh, in_specs=specs,
                  out_specs=(PartitionSpec("core"),) * n_outs,
                  check_rep=False),
        donate_argnums=donate, keep_unused=True)

    def run(feeds):
        """feeds: dict name -> full concatenated array [NCORES*dim0, ...].
        Returns dict name -> full concatenated output array."""
        concat_in = [feeds[name] for name in in_names]
        concat_zeros = [
            _np.zeros((NCORES * s[0],) + tuple(s[1:]), dt)
            for (s, dt) in zero_shapes
        ]
        out_arrs = sharded(*concat_in, *concat_zeros)
        return {name: _np.asarray(out_arrs[i])
                for i, name in enumerate(out_names)}

    # exposed for benchmarking (test.py)
    run._sharded = sharded
    run._in_names = in_names
    run._zero_shapes = zero_shapes
    return run


def _get_runner(caps, mode, has_bias):
    key = (caps, mode, has_bias)
    if key not in _CACHE:
        nc = _build(caps, mode, has_bias)
        _CACHE[key] = (nc, _make_runner(nc))
    return _CACHE[key]


def _route(x, Wg, bg, k):
    """Replicates the reference selector in fp32: softmax gate, top-k
    (stable, ties to lower index like jax.lax.top_k), aux loss."""
    logits = x @ Wg + bg
    m = logits.max(-1, keepdims=True)
    p = np.exp(logits - m)
    gate = p / p.sum(-1, keepdims=True)
    idx = np.argsort(-gate, axis=-1, kind="stable")[:, :k]      # [B, k]
    vals = np.take_along_axis(gate, idx, axis=-1)               # [B, k]
    row_sum = gate.sum(-1)
    aux = (np.var(row_sum) / (np.mean(row_sum) ** 2 + np.float32(1e-10)))
    return idx, vals, np.float32(aux)


def _split_into(dst, view, spec, F):
    """Write `view` [NCORES, 128, F] (fp32, any strides) into dst
    [NCORES*128, nsplit*F] as hi (and lo residual for split modes)."""
    d = dst.reshape(NCORES, 128, -1)
    d[:, :, :F] = view                      # cast fp32 -> device dtype
    if spec["nsplit"] == 2:
        d[:, :, F:] = view - d[:, :, :F].astype(np.float32)


def _prepare(x, Wg, bg, We, be, k, mode):
    """Route + dispatch: returns (caps, has_bias, feeds, dest, inv, aux)."""
    spec = _mode_spec(mode)
    idx, vals, aux = _route(x, Wg, bg, k)

    ef = idx.ravel()
    wf = vals.ravel()
    tf = np.repeat(np.arange(B), k)
    order = np.argsort(ef, kind="stable")
    counts = np.bincount(ef, minlength=E)

    # Load-sorted slot assignment: expert with load-rank r goes to core r%8,
    # slot r//8; slot j's capacity is the max token count in rank octile j
    # (rounded to 16), so every core compiles to the same block structure.
    rank_order = np.argsort(-counts, kind="stable")      # expert ids by load
    caps = tuple(int(max(16, -(-counts[rank_order[NCORES * j]] // 16) * 16))
                 for j in range(EPC))
    Cj = np.array(caps)
    CT = int(Cj.sum())
    ROFF = np.concatenate([[0], np.cumsum(Cj)]).astype(np.int64)

    core_of = np.empty(E, dtype=np.int64)
    slot_of = np.empty(E, dtype=np.int64)
    core_of[rank_order] = np.arange(E) % NCORES
    slot_of[rank_order] = np.arange(E) // NCORES

    starts = np.zeros(E, dtype=np.int64)
    starts[1:] = np.cumsum(counts)[:-1]
    base = core_of * CT + ROFF[slot_of]                  # per-expert row base
    es = ef[order]
    dest = base[es] + (np.arange(B * k) - starts[es])    # unique global rows
    inv = np.argsort(order, kind="stable")

    # Tokens are PRE-SCALED by their gate weight (w >= 0 so
    # relu((w*x)@W + w*be) == w*relu(x@W + be)); padding rows stay zero.
    Xg = np.zeros((NCORES * CT, NIN), dtype=np.float32)
    Xg[dest] = x[tf[order]] * wf[order][:, None]
    cwg = np.zeros(NCORES * CT, dtype=np.float32)
    cwg[dest] = wf[order]

    npdt = spec["npdt"]
    ns = spec["nsplit"]
    XF = KCH * CT
    WF = EPC * KCH * NOUT

    # swizzle straight into the concat-ready device feeds
    # x: [core][p (nin within chunk)][slot][kc][token], token-granular slots
    xsw = np.empty((NCORES, 128, XF), np.float32)
    Xg3 = Xg.reshape(NCORES, CT, NIN)
    for j in range(EPC):
        xsw[:, :, KCH * ROFF[j]:KCH * ROFF[j + 1]] = (
            Xg3[:, ROFF[j]:ROFF[j + 1], :]
            .reshape(NCORES, Cj[j], KCH, 128).transpose(0, 3, 2, 1)
            .reshape(NCORES, 128, KCH * Cj[j]))
    xd = np.empty((NCORES * 128, ns * XF), npdt)
    _split_into(xd, xsw, spec, XF)

    # w: [core][p][slot][kc][n]; core c's slot j holds expert rank_order[8j+c]
    eid = rank_order.reshape(EPC, NCORES).T              # [core, slot]
    wd = np.empty((NCORES * 128, ns * WF), npdt)
    _split_into(wd, We[eid].reshape(NCORES, EPC, KCH, 128, NOUT)
                .transpose(0, 3, 1, 2, 4).reshape(NCORES, 128, WF), spec, WF)

    has_bias = bool(np.any(be))
    feeds = {"xd": xd, "wd": wd}
    if has_bias:
        feeds["cw"] = np.ascontiguousarray(cwg.reshape(NCORES, CT))
        feeds["be"] = np.ascontiguousarray(
            be[eid].reshape(NCORES, EPC * NOUT))   # [core][1, e*n] rows
    return caps, has_bias, feeds, dest, inv, aux, CT


def kernel(x, Wg, bg, We, be, k):
    x = np.ascontiguousarray(np.asarray(x, dtype=np.float32))
    Wg = np.asarray(Wg, dtype=np.float32)
    bg = np.asarray(bg, dtype=np.float32)
    We = np.ascontiguousarray(np.asarray(We, dtype=np.float32))
    be = np.ascontiguousarray(np.asarray(be, dtype=np.float32))
    k = int(k)

    caps, has_bias, feeds, dest, inv, aux, CT = _prepare(x, Wg, bg, We, be,
                                                        k, MODE)
    _, run = _get_runner(caps, MODE, has_bias)
    results = run(feeds)

    # out is [NCORES*NOUT, CT] with nout on rows (weights were stationary)
    Y3 = results["out"].reshape(NCORES, NOUT, CT)
    d = dest[inv]                                       # token-major slots
    slot_rows = Y3[d // CT, :, d % CT]                  # [B*k, NOUT]
    out = slot_rows.reshape(B, k, NOUT).sum(axis=1, dtype=np.float32)
    return out, aux
